# revision 43
# baseline (speedup 1.0000x reference)
"""ConvLSTM (T=16, B=4, C=32, HID=64, 64x64, 3x3 convs) on 8 Trainium2 cores.

Decomposition: 8 cores = batch(4) x H-halves(2). Each core owns 32 output rows
and recomputes a shrinking halo (rows 0..47-t at step t) so NO inter-core
communication is needed. The bottom-half cores get vertically flipped inputs
(and dy-flipped conv weights) so all 8 cores run the identical SPMD program.

Conv-as-matmul with dense tap packing: the 9 taps x 32ch (x2h) and 9 taps x
64ch (h2h) of the two 3x3 convs are packed into the 128-partition contraction
dim as shifted copies of the padded image, reaching the floor of 7 matmuls
per 128-out-channel half per 512-pixel chunk:
    X1 = x taps (0,0)(0,1)(0,2)(1,0)   [4x32 rows, shifts 0,1,2,66]
    X2 = x taps (1,1)(1,2)(2,0)(2,1)   [shifts 67,68,132,133]
    HA @ dy*66, dy=0,1,2 = h taps (dy,0)(dy,1)   [2x64 rows, shifts 0,1]
    HB2 = h taps (0,2)(2,2)            [shifts 2,134]
    C96 = h tap (1,2) + x tap (2,2)    [96 rows; cmb holds a copy of tap
          (0,2) on parts 0:64 and the x block shifted +66 on parts 64:96,
          both read at row y0+1]
x blocks are DMA'd from DRAM at shifted offsets (free); h blocks are written
by 5 whole-row SBUF DMAs per 2-chunk group. All matmuls accumulate at base
partition 0 (mixed-base accumulation hangs the HW). MM_DT selects matmul
precision: bf16 (fast) or float32r (~20x more precise, ~1.7x slower PE).

Perf round 1 (479us -> 398us): padded 66-pitch staging ring for whole-row
halo DMAs (1 descriptor/partition); [i;o]/[g;f] gate packing for merged
sigmoid + ACT cross-partition-base writes.

Perf round 2 (398us -> ~326us):
 - the 64-row h-tap-(1,2) and 32-row x-tap-(2,2) matmuls merge into one
   96-row MM (14 MMs/chunk, the contraction floor of ceil(864/128) x 2):
   cmb parts 0:64 get a copy of the tap-(0,2) halo rows (extra per-chunk
   DMA on the lightly-loaded gpsimd queue) and the cmb x DMA lands
   shifted +66 so both halves read at row y0+1.
 - startup ramp fix: the step-1 x loads used to queue behind ~25us of
   init memsets on gpsimd, leaving a ~19us PE-idle that re-throttled HAM
   (PE clock gate) back to 1.2 GHz for step 1. Now cmb memsets go first
   on the vector queue, the x loads are emitted immediately after on
   gpsimd, remaining memsets split vector/gpsimd, and the 16-MM warmup
   reads a locally-memset dummy tile so it starts at ~3us instead of
   waiting ~13us for a weight DMA (first real MM ~15us, was ~25us).
 - routing the i*g mul to the gpsimd Pool engine or halo DMAs onto a
   6-deep sync queue regressed to 513us (Pool tensor ops are far slower
   on HW than the cost model's 0.42 efficiency suggests); bf16 gate
   tiles (for the DVE all-2-byte 2x mode) gained nothing and cost
   precision (1.0e-2) -- both reverted.

Negative results (all reverted, kept here so they are not retried): every
scheme that delays or coarsens the per-chunk tail chain loses big. The
per-chunk chain [io-sigmoid -> g/f ACT -> c-update DVEs -> tanh(c) ->
h-staging mul -> 4-5 halo DMAs] is ~6.5us and the next step's matmuls
wait on exactly these halo writes ~4 wavefront slots later (~13us runway,
roughly half of it eaten by ACT/DVE/Sync queue contention):
 - weight-outer MM emission over step-local 2-chunk groups with [128,1024]
   PSUM tiles + 1024-wide ACT/DVE tails: 506us (PE stalled ~6us every
   step boundary, HAM re-throttled, throttle_active 220us).
 - weight-outer pairing of wavefront-adjacent (step t, step t+1) chunks:
   488us (the fresh-dependency member stalls its whole pair).
 - intra-step adjacent pairing with per-chunk tails: 404us.
 - batching tanh(c)+staging+halo DMAs per 2-chunk group (singles MMs):
   387us. LAG=6/7 instead of 5: no effect (queue latency, not emission
   order, is binding).
The ~100ns LDWEIGHTS stall on ~4 MMs/chunk (single background weight
buffer, single-use weights) is the price of the fine-grained pipeline;
all attempts to amortize it cost more than the ~37us it wastes.

fp8 DoubleRow (2x PE) is numerically dead here: bf16 matmul quantization
already accounts for ~8.8e-3 of the 2e-2 rel-err budget and fp8e4m3 is
16x coarser. A pairwise-AllReduce halo exchange (to drop the shrinking
halo recompute, -23% work) measures 6.5-7.5us per CC op + ~5us of
bounce DMAs/sem props -- longer than the ~13us/step of hideable slack,
so it was not pursued.
"""
import sys
import os

for _p in ("/opt/trn_rl_repo", "/root/.axon_site"):
    if _p not in sys.path and os.path.isdir(_p):
        sys.path.append(_p)

import numpy as np

T, B, C, H, W, HID = 16, 4, 32, 64, 64, 64
HP, WP = 49, 66          # padded per-core image: 48 data rows + 1 top pad, 64+2 cols
FLAT = HP * WP           # 3234
NR = 8                   # output rows per chunk (N = NR*64 = 512 <= PSUM bank)
XLEN = 3100              # per-block x DMA length (covers max read f=3099)

MM_DT = os.environ.get("KLSTM_MM_DT", "bf16")   # "bf16" | "f32r"

X_TAPS_A = [(0, 0), (0, 1), (0, 2), (1, 0)]
X_TAPS_B = [(1, 1), (1, 2), (2, 0), (2, 1)]

_CACHE = {}


def _build_program():
    import concourse.mybir as mybir
    import concourse.tile as tile
    from concourse import bacc

    f32 = mybir.dt.float32
    dtm = mybir.dt.bfloat16 if MM_DT == "bf16" else mybir.dt.float32r

    nc = bacc.Bacc("TRN2", target_bir_lowering=False, debug=False, num_devices=8)

    xp_d = nc.dram_tensor("xp", [T, C, FLAT], dtm, kind="ExternalInput")
    wx1_d = nc.dram_tensor("wx1", [128, 256], dtm, kind="ExternalInput")
    wx2_d = nc.dram_tensor("wx2", [128, 256], dtm, kind="ExternalInput")
    wa_d = nc.dram_tensor("wa", [128, 768], dtm, kind="ExternalInput")
    wb2_d = nc.dram_tensor("wb2", [128, 256], dtm, kind="ExternalInput")
    wc_d = nc.dram_tensor("wc", [96, 256], dtm, kind="ExternalInput")
    bias_d = nc.dram_tensor("bias", [128, 2], f32, kind="ExternalInput")
    out_d = nc.dram_tensor("out", [T, HID, 32 * 66], f32, kind="ExternalOutput")

    Sigmoid = mybir.ActivationFunctionType.Sigmoid
    Tanh = mybir.ActivationFunctionType.Tanh

    with tile.TileContext(nc) as tc:
        with tc.tile_pool(name="const", bufs=1) as constp, \
             tc.tile_pool(name="xpool", bufs=4 if MM_DT == "bf16" else 2) as xpool, \
             tc.tile_pool(name="hpool", bufs=1) as hpool, \
             tc.tile_pool(name="cpool", bufs=1) as cpool, \
             tc.tile_pool(name="psum", bufs=4, space="PSUM") as psum, \
             tc.tile_pool(name="ifsp", bufs=6) as ifsp, \
             tc.tile_pool(name="smallp", bufs=6) as smallp:

            wx1_s = constp.tile([128, 256], dtm)
            wx2_s = constp.tile([128, 256], dtm)
            wa_s = constp.tile([128, 768], dtm)
            wb2_s = constp.tile([128, 256], dtm)
            wc_s = constp.tile([96, 256], dtm)
            bias_s = constp.tile([128, 2], f32)
            for s_, d_ in [(wa_s, wa_d), (wx1_s, wx1_d), (wx2_s, wx2_d),
                           (wb2_s, wb2_d), (wc_s, wc_d), (bias_s, bias_d)]:
                nc.sync.dma_start(s_[:], d_[:])

            # HAM warmup: junk matmuls during the startup ramp (while the
            # first x tiles load) so step 1's matmuls run at 2.4 GHz. The
            # operand is a locally-memset dummy (vector queue, ~1us) so the
            # warmup starts at ~3us instead of waiting ~13us for a weight
            # DMA through the sync queue; 16 cold MMs (~7us) land exactly
            # when the step-1 x tiles arrive (~8us).
            dummy = constp.tile([128, 512], dtm)
            nc.vector.memset(dummy[:] if MM_DT == "bf16"
                             else dummy[:].bitcast(f32), 0.0)
            wrm = psum.tile([128, 512], f32, tag="psA", name="warm")
            for _ in range(16):
                nc.tensor.matmul(wrm[:, 0:512], dummy[:, 0:128],
                                 dummy[:, 0:512], start=True, stop=True)

            # ping-pong h tiles (shifted partition blocks, see module docstring)
            hA = [hpool.tile([128, FLAT], dtm, tag=f"hA{i}", name=f"hA{i}")
                  for i in range(2)]
            hB2 = [hpool.tile([128, FLAT], dtm, tag=f"hB2{i}", name=f"hB2{i}")
                   for i in range(2)]
            # cmb: parts 0-63 copy of h tap (0,2) rows, parts 64-95 x tap
            # (2,2) shifted +66; both read at row y0+1 by the 96-row MM.
            # 3-deep so step t+1's x-load never waits on step t-1's matmuls
            cmb = [hpool.tile([96, FLAT], dtm, tag=f"cmb{i}", name=f"cmb{i}")
                   for i in range(3)]
            # (the former 66-pitch hout staging ring is gone: the h mul now
            # writes its padded-row layout directly into hA[0:64])
            # init order matters for the startup ramp: cmb memsets go first
            # (vector queue) because load_x(1) writes into cmb[1]; the x
            # loads (gpsimd) are emitted next -- see below -- so they are
            # NOT stuck behind ~25us of memsets (that startup PE-idle
            # re-throttled HAM and ran step 1 cold); remaining memsets
            # split across vector/gpsimd.
            def _ms(eng, t_):
                eng.memset(
                    t_[:] if MM_DT == "bf16" else t_[:].bitcast(f32), 0.0)
            for t_ in cmb:
                _ms(nc.vector, t_)

            c_s = cpool.tile([64, 47 * 64], f32)

            def load_x(t):
                # x tiles for step t: TA (4 blocks), TB (4 blocks), cmb x-block
                # (issued from the mostly-idle gpsimd queue, ~one step ahead)
                xa = xpool.tile([128, FLAT], dtm, tag="xa", name="xa")
                xb = xpool.tile([128, FLAT], dtm, tag="xb", name="xb")
                for b3, (dy, dx) in enumerate(X_TAPS_A):
                    s = dy * WP + dx
                    nc.gpsimd.dma_start(xa[32 * b3:32 * b3 + 32, 0:XLEN],
                                        xp_d[t - 1, :, s:s + XLEN])
                for b3, (dy, dx) in enumerate(X_TAPS_B):
                    s = dy * WP + dx
                    nc.gpsimd.dma_start(xb[32 * b3:32 * b3 + 32, 0:XLEN],
                                        xp_d[t - 1, :, s:s + XLEN])
                nc.gpsimd.dma_start(cmb[t % 3][64:96, 66:66 + XLEN],
                                    xp_d[t - 1, :, 134:134 + XLEN])
                return xa, xb

            def rv(tile_ap):
                return tile_ap.rearrange("p (y x) -> p y x", x=WP)

            xload = {1: load_x(1)}
            for t_ in hA + hB2:
                _ms(nc.vector, t_)
            ctx = {}

            def step_ctx(t):
                if t not in ctx:
                    if t + 1 <= T and t + 1 not in xload:
                        xload[t + 1] = load_x(t + 1)
                    xa, xb = xload[t]
                    ctx[t] = dict(
                        xav=rv(xa[:]), xbv=rv(xb[:]),
                        hAv=rv(hA[(t - 1) % 2][:]),
                        hB2v=rv(hB2[(t - 1) % 2][:]),
                        cmbv=rv(cmb[t % 3][:]),
                        hAc=hA[t % 2], hB2c=hB2[t % 2],
                        cmbn=cmb[(t + 1) % 3],
                    )
                return ctx[t]

            def weight_seq(t, cx, h):
                # the 7 stationary weights of one output-half, each with its
                # rhs builder. Step 1 emits the h matmuls too -- they read
                # the memset-zero h tiles (exact, h0 = 0) so t=1's MM blocks
                # are full-sized: the t=1->t=2 wavefront then has steady-
                # state runway instead of a ~7us PE bubble + HAM re-throttle
                hs = h * 128
                seq = [
                    (wx1_s[:, hs:hs + 128],
                     lambda y0, nr: cx["xav"][:, y0:y0 + nr, 0:64]),
                    (wx2_s[:, hs:hs + 128],
                     lambda y0, nr: cx["xbv"][:, y0:y0 + nr, 0:64]),
                ]
                seq += [(wa_s[:, (dy * 2 + h) * 128:(dy * 2 + h + 1) * 128],
                         lambda y0, nr, dy=dy:
                         cx["hAv"][:, y0 + dy:y0 + dy + nr, 0:64])
                        for dy in range(3)]
                seq += [(wb2_s[:, hs:hs + 128],
                         lambda y0, nr: cx["hB2v"][:, y0:y0 + nr, 0:64])]
                seq += [(wc_s[0:96, hs:hs + 128],
                         lambda y0, nr:
                         cx["cmbv"][0:96, y0 + 1:y0 + 1 + nr, 0:64])]
                return seq

            def front_pair(pair):
                # weight-outer emission over a pair of wavefront-adjacent
                # chunks (same weights for all t>=2): each stationary weight
                # issues 2 back-to-back MMs so the next LDWEIGHTS always has
                # a full MM-stream shadow to hide in.
                for job in pair:
                    job["pa"] = psum.tile([128, 512], f32, tag="psA",
                                          name="psA")
                    job["pb"] = psum.tile([128, 512], f32, tag="psB",
                                          name="psB")
                    job["cx"] = step_ctx(job["t"])
                for h in range(2):
                    seqs = [weight_seq(job["t"], job["cx"], h) for job in pair]
                    nw = max(len(s) for s in seqs)
                    for wi in range(nw):
                        for job, seq in zip(pair, seqs):
                            if wi >= len(seq):
                                continue
                            lhsT, rhs_of = seq[wi]
                            y0, nr = job["y0"], job["nr"]
                            pt = job["pa"] if h == 0 else job["pb"]
                            nc.tensor.matmul(
                                pt[:, :nr * 64], lhsT, rhs_of(y0, nr),
                                start=(wi == 0), stop=(wi == len(seq) - 1))

            def tail_chunk(job):
                # per-chunk gate activations + c update (short dependency
                # chain, fine-grained pipelining)
                t, y0, nr = job["t"], job["y0"], job["nr"]
                pa, pb = job["pa"], job["pb"]
                N = nr * 64
                # pa = [i; o] -> one merged sigmoid; pb = [g; f]
                io = ifsp.tile([128, 512], f32, tag="io", name="io")
                nc.scalar.activation(io[:, :N], pa[0:128, :N], Sigmoid,
                                     bias=bias_s[0:128, 0:1])
                gt = smallp.tile([64, 512], f32, tag="gt")
                nc.scalar.activation(gt[:, :N], pb[0:64, :N], Tanh,
                                     bias=bias_s[0:64, 1:2])
                c_sl = c_s[:, y0 * 64:y0 * 64 + N]
                if t == 1:
                    nc.vector.tensor_mul(c_sl, io[0:64, :N], gt[:, :N])
                else:
                    fs_ = ifsp.tile([64, 512], f32, tag="fs", name="fs_")
                    nc.scalar.activation(fs_[:, :N], pb[64:128, :N],
                                         Sigmoid, bias=bias_s[64:128, 1:2])
                    t1 = smallp.tile([64, 512], f32, tag="t1")
                    nc.vector.tensor_mul(t1[:, :N], io[0:64, :N], gt[:, :N])
                    nc.vector.tensor_mul(c_sl, fs_[:, :N], c_sl)
                    nc.vector.tensor_add(c_sl, c_sl, t1[:, :N])
                # tanh(c) cross-written to parts 64-127, next to o
                tc_ = smallp.tile([128, 512], f32, tag="tc")
                nc.scalar.activation(tc_[64:128, :N], c_sl, Tanh)
                cx = job["cx"]
                L = nr * 66
                d0 = (y0 + 1) * 66
                hAc, hB2c, cmbn = cx["hAc"], cx["hB2c"], cx["cmbn"]
                if t < T or y0 < 32:
                    # the h staging mul writes DIRECTLY into hA[0:64] (its
                    # layout -- row r at 66-pitch, cols 1:65 -- IS the
                    # tap-(dy,0) halo image, W-pad cols stay memset-0):
                    # the most latency-critical halo write needs no DMA at
                    # all, and the other shifted copies source from here.
                    hA3 = hAc[0:64].rearrange("p (y x) -> p y x", x=66)
                    o3 = io[64:128, :N].rearrange("p (y x) -> p y x", x=64)
                    t3 = tc_[64:128, :N].rearrange("p (y x) -> p y x", x=64)
                    nc.vector.tensor_mul(
                        hA3[:, y0 + 1:y0 + 1 + nr, 1:65], o3, t3)
                if t < T:
                    # whole-row contiguous shifted copies out of hA[0:64]
                    # (1 desc/partition); trailing stale elements land only
                    # in never-read pad columns (>=64) of the destinations.
                    # The cmb copy (for the merged 96-row MM) rides the
                    # lighter gpsimd queue.
                    # +2-shift transfers are trimmed 2 elements short so the
                    # source never touches the next chunk's row region (its
                    # mul writes col 1 there -- a would-be serialization);
                    # the dropped destination bytes are pad cols 64/65.
                    hsrc = hAc[0:64]
                    nc.sync.dma_start(hAc[64:128, d0:d0 + L],
                                      hsrc[:, d0 + 1:d0 + 1 + L])
                    nc.sync.dma_start(hB2c[0:64, d0:d0 + L - 2],
                                      hsrc[:, d0 + 2:d0 + L])
                    nc.gpsimd.dma_start(cmbn[0:64, d0:d0 + L - 2],
                                        hsrc[:, d0 + 2:d0 + L])
                    if y0 == 0:
                        nc.sync.dma_start(hB2c[64:128, 0:L - 68],
                                          hsrc[:, d0 + 68:d0 + L])
                    else:
                        d4 = (y0 - 1) * 66
                        nc.sync.dma_start(hB2c[64:128, d4:d4 + L - 2],
                                          hsrc[:, d0 + 2:d0 + L])
                if y0 < 32:
                    # out rows via casting SWDGE DMA straight from the bf16
                    # halo rows (padded 66-el layout; host strips)
                    src = (hAc[0:64, d0:d0 + L] if MM_DT == "bf16"
                           else hAc[0:64, d0:d0 + L].bitcast(f32))
                    nc.gpsimd.dma_start(
                        out_d[t - 1, :, y0 * 66:y0 * 66 + L], src)

            # cross-step wavefront: step t+1's chunk q is emitted alongside
            # step t's chunk q+LAG, so the next step's matmuls depend only on
            # halo writes that are already emitted (their true inputs) and
            # the PE never stalls at a step boundary. MM emission is fused
            # over INTRA-step adjacent chunk pairs (at the even chunk's
            # wavefront slot): both members' inputs are >=4 slots mature, so
            # the pair never waits (pairing a step-t chunk with a step-t+1
            # chunk here stalled the PE ~5us per pair on fresh halo writes).
            LAG = int(os.environ.get("KLSTM_LAG", "5"))
            pairs = []
            for t in range(1, T + 1):
                R = 48 - t
                nch = (R + NR - 1) // NR
                chunks = [dict(t=t, q=q, y0=q * NR, nr=min(NR, R - q * NR))
                          for q in range(nch)]
                PW = int(os.environ.get("KLSTM_PAIR", "1"))
                for k in range(0, nch, PW):
                    pr = chunks[k:k + PW]
                    pairs.append((LAG * (t - 1) + k, t, pr))
            pairs.sort(key=lambda p: (p[0], p[1]))
            for _, t, pr in pairs:
                front_pair(pr)
                for job in pr:
                    tail_chunk(job)
    nc.compile()
    return nc


def _host_prep(x, w_x2h, b_x2h, w_h2h, b_h2h):
    """Build the 8 per-core input maps."""
    import ml_dtypes
    np_dtm = ml_dtypes.bfloat16 if MM_DT == "bf16" else np.float32

    x = np.ascontiguousarray(np.asarray(x, np.float32))
    w_x2h = np.asarray(w_x2h, np.float32)
    b_x2h = np.asarray(b_x2h, np.float32)
    w_h2h = np.asarray(w_h2h, np.float32)
    b_h2h = np.asarray(b_h2h, np.float32)

    # gate-channel permutation: [i, o, g, f] so psum half0=[i;o], half1=[g;f]
    order = np.r_[0:64, 192:256, 128:192, 64:128]

    bias = np.zeros((128, 2), np.float32)
    bsum = (b_x2h + b_h2h)[order]
    bias[:, 0] = bsum[0:128]
    bias[:, 1] = bsum[128:256]

    in_maps = []
    packed_w = {}
    for parity in range(2):
        wx_f = (w_x2h if parity == 0 else w_x2h[:, :, ::-1, :])[order]
        wh_f = (w_h2h if parity == 0 else w_h2h[:, :, ::-1, :])[order]
        wx1 = np.zeros((128, 2, 128), np.float32)
        wx2 = np.zeros((128, 2, 128), np.float32)
        wa = np.zeros((128, 3, 2, 128), np.float32)
        wb2 = np.zeros((128, 2, 128), np.float32)
        wc = np.zeros((96, 2, 128), np.float32)
        for hh in range(2):
            oc = slice(hh * 128, (hh + 1) * 128)
            for b3, (dy, dx) in enumerate(X_TAPS_A):
                wx1[32 * b3:32 * b3 + 32, hh, :] = wx_f[oc, :, dy, dx].T
            for b3, (dy, dx) in enumerate(X_TAPS_B):
                wx2[32 * b3:32 * b3 + 32, hh, :] = wx_f[oc, :, dy, dx].T
            for dy in range(3):
                for b3 in range(2):
                    wa[64 * b3:64 * b3 + 64, dy, hh, :] = wh_f[oc, :, dy, b3].T
            wb2[0:64, hh, :] = wh_f[oc, :, 0, 2].T
            wb2[64:128, hh, :] = wh_f[oc, :, 2, 2].T
            wc[0:64, hh, :] = wh_f[oc, :, 1, 2].T
            wc[64:96, hh, :] = wx_f[oc, :, 2, 2].T
        packed_w[parity] = tuple(
            np.ascontiguousarray(a.reshape(a.shape[0], -1).astype(np_dtm))
            for a in (wx1, wx2, wa, wb2, wc))

    for core in range(8):
        b, parity = core // 2, core % 2
        xv = x[:, b]
        if parity == 1:
            xv = xv[:, :, ::-1, :]
        xp = np.zeros((T, C, HP, WP), np.float32)
        xp[:, :, 1:49, 1:65] = xv[:, :, 0:48, :]
        wx1, wx2, wa, wb2, wc = packed_w[parity]
        in_maps.append({
            "xp": np.ascontiguousarray(xp.reshape(T, C, FLAT).astype(np_dtm)),
            "wx1": wx1, "wx2": wx2, "wa": wa, "wb2": wb2, "wc": wc,
            "bias": bias,
        })
    return in_maps


def kernel(x, w_x2h, b_x2h, w_h2h, b_h2h, _trace=False, _tmpdir=None):
    from concourse.bass_utils import run_bass_kernel_spmd

    if "nc" not in _CACHE:
        _CACHE["nc"] = _build_program()
    nc = _CACHE["nc"]

    in_maps = _host_prep(x, w_x2h, b_x2h, w_h2h, b_h2h)
    kw = {}
    if _trace:
        kw = dict(trace=True, tmpdir=_tmpdir)
    res = run_bass_kernel_spmd(nc, in_maps, core_ids=list(range(8)), **kw)

    full = np.zeros((T, B, HID, H, W), np.float32)
    for core in range(8):
        b, parity = core // 2, core % 2
        out = res.results[core]["out"].reshape(T, HID, 32, 66)[:, :, :, 1:65]
        if parity == 0:
            full[:, b, :, 0:32] = out
        else:
            full[:, b, :, 32:64] = out[:, :, ::-1, :]
    if _trace:
        return full, res
    return full


# revision 44
# speedup vs baseline: 1.0188x; 1.0188x over previous
"""ConvLSTM (T=16, B=4, C=32, HID=64, 64x64, 3x3 convs) on 8 Trainium2 cores.

Decomposition: 8 cores = batch(4) x H-halves(2). Each core owns 32 output rows
and recomputes a shrinking halo (rows 0..47-t at step t) so NO inter-core
communication is needed. The bottom-half cores get vertically flipped inputs
(and dy-flipped conv weights) so all 8 cores run the identical SPMD program.

Conv-as-matmul with dense tap packing: the 9 taps x 32ch (x2h) and 9 taps x
64ch (h2h) of the two 3x3 convs are packed into the 128-partition contraction
dim as shifted copies of the padded image, reaching the floor of 7 matmuls
per 128-out-channel half per 512-pixel chunk:
    X1 = x taps (0,0)(0,1)(0,2)(1,0)   [4x32 rows, shifts 0,1,2,66]
    X2 = x taps (1,1)(1,2)(2,0)(2,1)   [shifts 67,68,132,133]
    HA @ dy*66, dy=0,1,2 = h taps (dy,0)(dy,1)   [2x64 rows, shifts 0,1]
    HB2 = h taps (0,2)(2,2)            [shifts 2,134]
    C96 = h tap (1,2) + x tap (2,2)    [96 rows; cmb holds a copy of tap
          (0,2) on parts 0:64 and the x block shifted +66 on parts 64:96,
          both read at row y0+1]
x blocks are DMA'd from DRAM at shifted offsets (free); h blocks are written
by 5 whole-row SBUF DMAs per 2-chunk group. All matmuls accumulate at base
partition 0 (mixed-base accumulation hangs the HW). MM_DT selects matmul
precision: bf16 (fast) or float32r (~20x more precise, ~1.7x slower PE).

Perf round 1 (479us -> 398us): padded 66-pitch staging ring for whole-row
halo DMAs (1 descriptor/partition); [i;o]/[g;f] gate packing for merged
sigmoid + ACT cross-partition-base writes.

Perf round 2 (398us -> ~326us):
 - the 64-row h-tap-(1,2) and 32-row x-tap-(2,2) matmuls merge into one
   96-row MM (14 MMs/chunk, the contraction floor of ceil(864/128) x 2):
   cmb parts 0:64 get a copy of the tap-(0,2) halo rows (extra per-chunk
   DMA on the lightly-loaded gpsimd queue) and the cmb x DMA lands
   shifted +66 so both halves read at row y0+1.
 - startup ramp fix: the step-1 x loads used to queue behind ~25us of
   init memsets on gpsimd, leaving a ~19us PE-idle that re-throttled HAM
   (PE clock gate) back to 1.2 GHz for step 1. Now cmb memsets go first
   on the vector queue, the x loads are emitted immediately after on
   gpsimd, remaining memsets split vector/gpsimd, and the 16-MM warmup
   reads a locally-memset dummy tile so it starts at ~3us instead of
   waiting ~13us for a weight DMA (first real MM ~15us, was ~25us).
 - routing the i*g mul to the gpsimd Pool engine or halo DMAs onto a
   6-deep sync queue regressed to 513us (Pool tensor ops are far slower
   on HW than the cost model's 0.42 efficiency suggests); bf16 gate
   tiles (for the DVE all-2-byte 2x mode) gained nothing and cost
   precision (1.0e-2) -- both reverted.

Negative results (all reverted, kept here so they are not retried): every
scheme that delays or coarsens the per-chunk tail chain loses big. The
per-chunk chain [io-sigmoid -> g/f ACT -> c-update DVEs -> tanh(c) ->
h-staging mul -> 4-5 halo DMAs] is ~6.5us and the next step's matmuls
wait on exactly these halo writes ~4 wavefront slots later (~13us runway,
roughly half of it eaten by ACT/DVE/Sync queue contention):
 - weight-outer MM emission over step-local 2-chunk groups with [128,1024]
   PSUM tiles + 1024-wide ACT/DVE tails: 506us (PE stalled ~6us every
   step boundary, HAM re-throttled, throttle_active 220us).
 - weight-outer pairing of wavefront-adjacent (step t, step t+1) chunks:
   488us (the fresh-dependency member stalls its whole pair).
 - intra-step adjacent pairing with per-chunk tails: 404us.
 - batching tanh(c)+staging+halo DMAs per 2-chunk group (singles MMs):
   387us. LAG=6/7 instead of 5: no effect (queue latency, not emission
   order, is binding).
The ~100ns LDWEIGHTS stall on ~4 MMs/chunk (single background weight
buffer, single-use weights) is the price of the fine-grained pipeline;
all attempts to amortize it cost more than the ~37us it wastes.

fp8 DoubleRow (2x PE) is numerically dead here: bf16 matmul quantization
already accounts for ~8.8e-3 of the 2e-2 rel-err budget and fp8e4m3 is
16x coarser. A pairwise-AllReduce halo exchange (to drop the shrinking
halo recompute, -23% work) measures 6.5-7.5us per CC op + ~5us of
bounce DMAs/sem props -- longer than the ~13us/step of hideable slack,
so it was not pursued.
"""
import sys
import os

for _p in ("/opt/trn_rl_repo", "/root/.axon_site"):
    if _p not in sys.path and os.path.isdir(_p):
        sys.path.append(_p)

import numpy as np

T, B, C, H, W, HID = 16, 4, 32, 64, 64, 64
HP, WP = 49, 66          # padded per-core image: 48 data rows + 1 top pad, 64+2 cols
FLAT = HP * WP           # 3234
NR = 8                   # output rows per chunk (N = NR*64 = 512 <= PSUM bank)
XLEN = 3100              # per-block x DMA length (covers max read f=3099)

MM_DT = os.environ.get("KLSTM_MM_DT", "bf16")   # "bf16" | "f32r"

X_TAPS_A = [(0, 0), (0, 1), (0, 2), (1, 0)]
X_TAPS_B = [(1, 1), (1, 2), (2, 0), (2, 1)]

_CACHE = {}


def _build_program():
    import concourse.mybir as mybir
    import concourse.tile as tile
    from concourse import bacc

    f32 = mybir.dt.float32
    dtm = mybir.dt.bfloat16 if MM_DT == "bf16" else mybir.dt.float32r

    nc = bacc.Bacc("TRN2", target_bir_lowering=False, debug=False, num_devices=8)

    xp_d = nc.dram_tensor("xp", [T, C, FLAT], dtm, kind="ExternalInput")
    wx1_d = nc.dram_tensor("wx1", [128, 256], dtm, kind="ExternalInput")
    wx2_d = nc.dram_tensor("wx2", [128, 256], dtm, kind="ExternalInput")
    wa_d = nc.dram_tensor("wa", [128, 768], dtm, kind="ExternalInput")
    wb2_d = nc.dram_tensor("wb2", [128, 256], dtm, kind="ExternalInput")
    wc_d = nc.dram_tensor("wc", [96, 256], dtm, kind="ExternalInput")
    bias_d = nc.dram_tensor("bias", [128, 2], f32, kind="ExternalInput")
    out_d = nc.dram_tensor("out", [T, HID, 32 * 66], f32, kind="ExternalOutput")

    Sigmoid = mybir.ActivationFunctionType.Sigmoid
    Tanh = mybir.ActivationFunctionType.Tanh

    with tile.TileContext(nc) as tc:
        with tc.tile_pool(name="const", bufs=1) as constp, \
             tc.tile_pool(name="xpool", bufs=4 if MM_DT == "bf16" else 2) as xpool, \
             tc.tile_pool(name="hpool", bufs=1) as hpool, \
             tc.tile_pool(name="cpool", bufs=1) as cpool, \
             tc.tile_pool(name="psum", bufs=4, space="PSUM") as psum, \
             tc.tile_pool(name="ifsp", bufs=6) as ifsp, \
             tc.tile_pool(name="smallp", bufs=6) as smallp:

            wx1_s = constp.tile([128, 256], dtm)
            wx2_s = constp.tile([128, 256], dtm)
            wa_s = constp.tile([128, 768], dtm)
            wb2_s = constp.tile([128, 256], dtm)
            wc_s = constp.tile([96, 256], dtm)
            bias_s = constp.tile([128, 2], f32)
            for s_, d_ in [(wa_s, wa_d), (wx1_s, wx1_d), (wx2_s, wx2_d),
                           (wb2_s, wb2_d), (wc_s, wc_d), (bias_s, bias_d)]:
                nc.sync.dma_start(s_[:], d_[:])

            # HAM warmup: junk matmuls during the startup ramp (while the
            # first x tiles load) so step 1's matmuls run at 2.4 GHz. The
            # operand is a locally-memset dummy (vector queue, ~1us) so the
            # warmup starts at ~3us instead of waiting ~13us for a weight
            # DMA through the sync queue; 16 cold MMs (~7us) land exactly
            # when the step-1 x tiles arrive (~8us).
            dummy = constp.tile([128, 512], dtm)
            nc.vector.memset(dummy[:] if MM_DT == "bf16"
                             else dummy[:].bitcast(f32), 0.0)
            wrm = psum.tile([128, 512], f32, tag="psA", name="warm")
            for _ in range(28):
                nc.tensor.matmul(wrm[:, 0:512], dummy[:, 0:128],
                                 dummy[:, 0:512], start=True, stop=True)

            # ping-pong h tiles (shifted partition blocks, see module docstring)
            hA = [hpool.tile([128, FLAT], dtm, tag=f"hA{i}", name=f"hA{i}")
                  for i in range(2)]
            hB2 = [hpool.tile([128, FLAT], dtm, tag=f"hB2{i}", name=f"hB2{i}")
                   for i in range(2)]
            # cmb: parts 0-63 copy of h tap (0,2) rows, parts 64-95 x tap
            # (2,2) shifted +66; both read at row y0+1 by the 96-row MM.
            # 3-deep so step t+1's x-load never waits on step t-1's matmuls
            cmb = [hpool.tile([96, FLAT], dtm, tag=f"cmb{i}", name=f"cmb{i}")
                   for i in range(3)]
            # (the former 66-pitch hout staging ring is gone: the h mul now
            # writes its padded-row layout directly into hA[0:64])
            # init order matters for the startup ramp: cmb memsets go first
            # (vector queue) because load_x(1) writes into cmb[1]; the x
            # loads (gpsimd) are emitted next -- see below -- so they are
            # NOT stuck behind ~25us of memsets (that startup PE-idle
            # re-throttled HAM and ran step 1 cold); remaining memsets
            # split across vector/gpsimd.
            def _ms(eng, t_):
                eng.memset(
                    t_[:] if MM_DT == "bf16" else t_[:].bitcast(f32), 0.0)
            for t_ in cmb:
                _ms(nc.vector, t_)

            c_s = cpool.tile([64, 47 * 64], f32)

            def load_x(t):
                # x tiles for step t: TA (4 blocks), TB (4 blocks), cmb x-block
                # (issued from the mostly-idle gpsimd queue, ~one step ahead)
                xa = xpool.tile([128, FLAT], dtm, tag="xa", name="xa")
                xb = xpool.tile([128, FLAT], dtm, tag="xb", name="xb")
                for b3, (dy, dx) in enumerate(X_TAPS_A):
                    s = dy * WP + dx
                    nc.gpsimd.dma_start(xa[32 * b3:32 * b3 + 32, 0:XLEN],
                                        xp_d[t - 1, :, s:s + XLEN])
                for b3, (dy, dx) in enumerate(X_TAPS_B):
                    s = dy * WP + dx
                    nc.gpsimd.dma_start(xb[32 * b3:32 * b3 + 32, 0:XLEN],
                                        xp_d[t - 1, :, s:s + XLEN])
                nc.gpsimd.dma_start(cmb[t % 3][64:96, 66:66 + XLEN],
                                    xp_d[t - 1, :, 134:134 + XLEN])
                return xa, xb

            def rv(tile_ap):
                return tile_ap.rearrange("p (y x) -> p y x", x=WP)

            xload = {1: load_x(1)}
            for t_ in hA + hB2:
                _ms(nc.vector, t_)
            ctx = {}

            def step_ctx(t):
                if t not in ctx:
                    if t + 1 <= T and t + 1 not in xload:
                        xload[t + 1] = load_x(t + 1)
                    xa, xb = xload[t]
                    ctx[t] = dict(
                        xav=rv(xa[:]), xbv=rv(xb[:]),
                        hAv=rv(hA[(t - 1) % 2][:]),
                        hB2v=rv(hB2[(t - 1) % 2][:]),
                        cmbv=rv(cmb[t % 3][:]),
                        hAc=hA[t % 2], hB2c=hB2[t % 2],
                        cmbn=cmb[(t + 1) % 3],
                    )
                return ctx[t]

            def weight_seq(t, cx, h):
                # the 7 (or 3 at t=1) stationary weights of one output-half,
                # each with its rhs builder
                hs = h * 128
                seq = [
                    (wx1_s[:, hs:hs + 128],
                     lambda y0, nr: cx["xav"][:, y0:y0 + nr, 0:64]),
                    (wx2_s[:, hs:hs + 128],
                     lambda y0, nr: cx["xbv"][:, y0:y0 + nr, 0:64]),
                ]
                if t > 1:
                    seq += [(wa_s[:, (dy * 2 + h) * 128:(dy * 2 + h + 1) * 128],
                             lambda y0, nr, dy=dy:
                             cx["hAv"][:, y0 + dy:y0 + dy + nr, 0:64])
                            for dy in range(3)]
                    seq += [(wb2_s[:, hs:hs + 128],
                             lambda y0, nr: cx["hB2v"][:, y0:y0 + nr, 0:64])]
                seq += [(wc_s[0:96, hs:hs + 128],
                         lambda y0, nr:
                         cx["cmbv"][0:96, y0 + 1:y0 + 1 + nr, 0:64])]
                return seq

            def front_pair(pair):
                # weight-outer emission over a pair of wavefront-adjacent
                # chunks (same weights for all t>=2): each stationary weight
                # issues 2 back-to-back MMs so the next LDWEIGHTS always has
                # a full MM-stream shadow to hide in.
                for job in pair:
                    job["pa"] = psum.tile([128, 512], f32, tag="psA",
                                          name="psA")
                    job["pb"] = psum.tile([128, 512], f32, tag="psB",
                                          name="psB")
                    job["cx"] = step_ctx(job["t"])
                for h in range(2):
                    seqs = [weight_seq(job["t"], job["cx"], h) for job in pair]
                    nw = max(len(s) for s in seqs)
                    for wi in range(nw):
                        for job, seq in zip(pair, seqs):
                            if wi >= len(seq):
                                continue
                            lhsT, rhs_of = seq[wi]
                            y0, nr = job["y0"], job["nr"]
                            pt = job["pa"] if h == 0 else job["pb"]
                            nc.tensor.matmul(
                                pt[:, :nr * 64], lhsT, rhs_of(y0, nr),
                                start=(wi == 0), stop=(wi == len(seq) - 1))

            def tail_chunk(job):
                # per-chunk gate activations + c update (short dependency
                # chain, fine-grained pipelining)
                t, y0, nr = job["t"], job["y0"], job["nr"]
                pa, pb = job["pa"], job["pb"]
                N = nr * 64
                # pa = [i; o] -> one merged sigmoid; pb = [g; f]
                io = ifsp.tile([128, 512], f32, tag="io", name="io")
                nc.scalar.activation(io[:, :N], pa[0:128, :N], Sigmoid,
                                     bias=bias_s[0:128, 0:1])
                gt = smallp.tile([64, 512], f32, tag="gt")
                nc.scalar.activation(gt[:, :N], pb[0:64, :N], Tanh,
                                     bias=bias_s[0:64, 1:2])
                c_sl = c_s[:, y0 * 64:y0 * 64 + N]
                if t == 1:
                    nc.vector.tensor_mul(c_sl, io[0:64, :N], gt[:, :N])
                else:
                    fs_ = ifsp.tile([64, 512], f32, tag="fs", name="fs_")
                    nc.scalar.activation(fs_[:, :N], pb[64:128, :N],
                                         Sigmoid, bias=bias_s[64:128, 1:2])
                    t1 = smallp.tile([64, 512], f32, tag="t1")
                    nc.vector.tensor_mul(t1[:, :N], io[0:64, :N], gt[:, :N])
                    nc.vector.tensor_mul(c_sl, fs_[:, :N], c_sl)
                    nc.vector.tensor_add(c_sl, c_sl, t1[:, :N])
                # tanh(c) cross-written to parts 64-127, next to o
                tc_ = smallp.tile([128, 512], f32, tag="tc")
                nc.scalar.activation(tc_[64:128, :N], c_sl, Tanh)
                cx = job["cx"]
                L = nr * 66
                d0 = (y0 + 1) * 66
                hAc, hB2c, cmbn = cx["hAc"], cx["hB2c"], cx["cmbn"]
                if t < T or y0 < 32:
                    # the h staging mul writes DIRECTLY into hA[0:64] (its
                    # layout -- row r at 66-pitch, cols 1:65 -- IS the
                    # tap-(dy,0) halo image, W-pad cols stay memset-0):
                    # the most latency-critical halo write needs no DMA at
                    # all, and the other shifted copies source from here.
                    hA3 = hAc[0:64].rearrange("p (y x) -> p y x", x=66)
                    o3 = io[64:128, :N].rearrange("p (y x) -> p y x", x=64)
                    t3 = tc_[64:128, :N].rearrange("p (y x) -> p y x", x=64)
                    nc.vector.tensor_mul(
                        hA3[:, y0 + 1:y0 + 1 + nr, 1:65], o3, t3)
                if t < T:
                    # whole-row contiguous shifted copies out of hA[0:64]
                    # (1 desc/partition); trailing stale elements land only
                    # in never-read pad columns (>=64) of the destinations.
                    # The cmb copy (for the merged 96-row MM) rides the
                    # lighter gpsimd queue.
                    # +2-shift transfers are trimmed 2 elements short so the
                    # source never touches the next chunk's row region (its
                    # mul writes col 1 there -- a would-be serialization);
                    # the dropped destination bytes are pad cols 64/65.
                    hsrc = hAc[0:64]
                    nc.sync.dma_start(hAc[64:128, d0:d0 + L],
                                      hsrc[:, d0 + 1:d0 + 1 + L])
                    nc.sync.dma_start(hB2c[0:64, d0:d0 + L - 2],
                                      hsrc[:, d0 + 2:d0 + L])
                    nc.gpsimd.dma_start(cmbn[0:64, d0:d0 + L - 2],
                                        hsrc[:, d0 + 2:d0 + L])
                    if y0 == 0:
                        nc.sync.dma_start(hB2c[64:128, 0:L - 68],
                                          hsrc[:, d0 + 68:d0 + L])
                    else:
                        d4 = (y0 - 1) * 66
                        nc.sync.dma_start(hB2c[64:128, d4:d4 + L - 2],
                                          hsrc[:, d0 + 2:d0 + L])
                if y0 < 32:
                    # out rows via casting SWDGE DMA straight from the bf16
                    # halo rows (padded 66-el layout; host strips)
                    src = (hAc[0:64, d0:d0 + L] if MM_DT == "bf16"
                           else hAc[0:64, d0:d0 + L].bitcast(f32))
                    nc.gpsimd.dma_start(
                        out_d[t - 1, :, y0 * 66:y0 * 66 + L], src)

            # cross-step wavefront: step t+1's chunk q is emitted alongside
            # step t's chunk q+LAG, so the next step's matmuls depend only on
            # halo writes that are already emitted (their true inputs) and
            # the PE never stalls at a step boundary. MM emission is fused
            # over INTRA-step adjacent chunk pairs (at the even chunk's
            # wavefront slot): both members' inputs are >=4 slots mature, so
            # the pair never waits (pairing a step-t chunk with a step-t+1
            # chunk here stalled the PE ~5us per pair on fresh halo writes).
            LAG = int(os.environ.get("KLSTM_LAG", "5"))
            pairs = []
            for t in range(1, T + 1):
                R = 48 - t
                nch = (R + NR - 1) // NR
                chunks = [dict(t=t, q=q, y0=q * NR, nr=min(NR, R - q * NR))
                          for q in range(nch)]
                PW = int(os.environ.get("KLSTM_PAIR", "1"))
                for k in range(0, nch, PW):
                    pr = chunks[k:k + PW]
                    pairs.append((LAG * (t - 1) + k, t, pr))
            pairs.sort(key=lambda p: (p[0], p[1]))
            for _, t, pr in pairs:
                front_pair(pr)
                for job in pr:
                    tail_chunk(job)
    nc.compile()
    return nc


def _host_prep(x, w_x2h, b_x2h, w_h2h, b_h2h):
    """Build the 8 per-core input maps."""
    import ml_dtypes
    np_dtm = ml_dtypes.bfloat16 if MM_DT == "bf16" else np.float32

    x = np.ascontiguousarray(np.asarray(x, np.float32))
    w_x2h = np.asarray(w_x2h, np.float32)
    b_x2h = np.asarray(b_x2h, np.float32)
    w_h2h = np.asarray(w_h2h, np.float32)
    b_h2h = np.asarray(b_h2h, np.float32)

    # gate-channel permutation: [i, o, g, f] so psum half0=[i;o], half1=[g;f]
    order = np.r_[0:64, 192:256, 128:192, 64:128]

    bias = np.zeros((128, 2), np.float32)
    bsum = (b_x2h + b_h2h)[order]
    bias[:, 0] = bsum[0:128]
    bias[:, 1] = bsum[128:256]

    in_maps = []
    packed_w = {}
    for parity in range(2):
        wx_f = (w_x2h if parity == 0 else w_x2h[:, :, ::-1, :])[order]
        wh_f = (w_h2h if parity == 0 else w_h2h[:, :, ::-1, :])[order]
        wx1 = np.zeros((128, 2, 128), np.float32)
        wx2 = np.zeros((128, 2, 128), np.float32)
        wa = np.zeros((128, 3, 2, 128), np.float32)
        wb2 = np.zeros((128, 2, 128), np.float32)
        wc = np.zeros((96, 2, 128), np.float32)
        for hh in range(2):
            oc = slice(hh * 128, (hh + 1) * 128)
            for b3, (dy, dx) in enumerate(X_TAPS_A):
                wx1[32 * b3:32 * b3 + 32, hh, :] = wx_f[oc, :, dy, dx].T
            for b3, (dy, dx) in enumerate(X_TAPS_B):
                wx2[32 * b3:32 * b3 + 32, hh, :] = wx_f[oc, :, dy, dx].T
            for dy in range(3):
                for b3 in range(2):
                    wa[64 * b3:64 * b3 + 64, dy, hh, :] = wh_f[oc, :, dy, b3].T
            wb2[0:64, hh, :] = wh_f[oc, :, 0, 2].T
            wb2[64:128, hh, :] = wh_f[oc, :, 2, 2].T
            wc[0:64, hh, :] = wh_f[oc, :, 1, 2].T
            wc[64:96, hh, :] = wx_f[oc, :, 2, 2].T
        packed_w[parity] = tuple(
            np.ascontiguousarray(a.reshape(a.shape[0], -1).astype(np_dtm))
            for a in (wx1, wx2, wa, wb2, wc))

    for core in range(8):
        b, parity = core // 2, core % 2
        xv = x[:, b]
        if parity == 1:
            xv = xv[:, :, ::-1, :]
        xp = np.zeros((T, C, HP, WP), np.float32)
        xp[:, :, 1:49, 1:65] = xv[:, :, 0:48, :]
        wx1, wx2, wa, wb2, wc = packed_w[parity]
        in_maps.append({
            "xp": np.ascontiguousarray(xp.reshape(T, C, FLAT).astype(np_dtm)),
            "wx1": wx1, "wx2": wx2, "wa": wa, "wb2": wb2, "wc": wc,
            "bias": bias,
        })
    return in_maps


def kernel(x, w_x2h, b_x2h, w_h2h, b_h2h, _trace=False, _tmpdir=None):
    from concourse.bass_utils import run_bass_kernel_spmd

    if "nc" not in _CACHE:
        _CACHE["nc"] = _build_program()
    nc = _CACHE["nc"]

    in_maps = _host_prep(x, w_x2h, b_x2h, w_h2h, b_h2h)
    kw = {}
    if _trace:
        kw = dict(trace=True, tmpdir=_tmpdir)
    res = run_bass_kernel_spmd(nc, in_maps, core_ids=list(range(8)), **kw)

    full = np.zeros((T, B, HID, H, W), np.float32)
    for core in range(8):
        b, parity = core // 2, core % 2
        out = res.results[core]["out"].reshape(T, HID, 32, 66)[:, :, :, 1:65]
        if parity == 0:
            full[:, b, :, 0:32] = out
        else:
            full[:, b, :, 32:64] = out[:, :, ::-1, :]
    if _trace:
        return full, res
    return full


# revision 48
# speedup vs baseline: 1.0330x; 1.0139x over previous
"""ConvLSTM (T=16, B=4, C=32, HID=64, 64x64, 3x3 convs) on 8 Trainium2 cores.

Decomposition: 8 cores = batch(4) x H-halves(2). Each core owns 32 output rows
and recomputes a shrinking halo (rows 0..47-t at step t) so NO inter-core
communication is needed. The bottom-half cores get vertically flipped inputs
(and dy-flipped conv weights) so all 8 cores run the identical SPMD program.

Conv-as-matmul with dense tap packing: the 9 taps x 32ch (x2h) and 9 taps x
64ch (h2h) of the two 3x3 convs are packed into the 128-partition contraction
dim as shifted copies of the padded image, reaching the floor of 7 matmuls
per 128-out-channel half per 512-pixel chunk:
    X1 = x taps (0,0)(0,1)(0,2)(1,0)   [4x32 rows, shifts 0,1,2,66]
    X2 = x taps (1,1)(1,2)(2,0)(2,1)   [shifts 67,68,132,133]
    HA @ dy*66, dy=0,1,2 = h taps (dy,0)(dy,1)   [2x64 rows, shifts 0,1]
    HB2 = h taps (0,2)(2,2)            [shifts 2,134]
    C96 = h tap (1,2) + x tap (2,2)    [96 rows; cmb holds a copy of tap
          (0,2) on parts 0:64 and the x block shifted +66 on parts 64:96,
          both read at row y0+1]
x blocks are DMA'd from DRAM at shifted offsets (free); h blocks are written
by 5 whole-row SBUF DMAs per 2-chunk group. All matmuls accumulate at base
partition 0 (mixed-base accumulation hangs the HW). MM_DT selects matmul
precision: bf16 (fast) or float32r (~20x more precise, ~1.7x slower PE).

Perf round 1 (479us -> 398us): padded 66-pitch staging ring for whole-row
halo DMAs (1 descriptor/partition); [i;o]/[g;f] gate packing for merged
sigmoid + ACT cross-partition-base writes.

Perf round 2 (398us -> ~326us):
 - the 64-row h-tap-(1,2) and 32-row x-tap-(2,2) matmuls merge into one
   96-row MM (14 MMs/chunk, the contraction floor of ceil(864/128) x 2):
   cmb parts 0:64 get a copy of the tap-(0,2) halo rows (extra per-chunk
   DMA on the lightly-loaded gpsimd queue) and the cmb x DMA lands
   shifted +66 so both halves read at row y0+1.
 - startup ramp fix: the step-1 x loads used to queue behind ~25us of
   init memsets on gpsimd, leaving a ~19us PE-idle that re-throttled HAM
   (PE clock gate) back to 1.2 GHz for step 1. Now cmb memsets go first
   on the vector queue, the x loads are emitted immediately after on
   gpsimd, remaining memsets split vector/gpsimd, and the 16-MM warmup
   reads a locally-memset dummy tile so it starts at ~3us instead of
   waiting ~13us for a weight DMA (first real MM ~15us, was ~25us).
 - routing the i*g mul to the gpsimd Pool engine or halo DMAs onto a
   6-deep sync queue regressed to 513us (Pool tensor ops are far slower
   on HW than the cost model's 0.42 efficiency suggests); bf16 gate
   tiles (for the DVE all-2-byte 2x mode) gained nothing and cost
   precision (1.0e-2) -- both reverted.

Perf round 3 (~326us -> ~302us):
 - the h staging mul (o * tanh(c)) writes DIRECTLY into hA[0:64]: its
   66-pitch padded-row layout IS the tap-(dy,0) halo image, so the most
   latency-critical halo write needs no DMA at all (saves ~1.9us of
   issue+transfer+sem-prop on the cross-step chain), the other shifted
   copies source from hA[0:64], the out DMA reads it too, and the whole
   staging ring disappears. The +2-shift copies are trimmed 2 elements so
   they never read the next chunk's row region (its mul writes col 1
   there -- Tile would serialize the next mul behind the copy); dropped
   destination bytes are only ever pad cols >=64, which no matmul reads.
 - also tried and reverted: emitting full 14-MM blocks at t=1 against the
   memset-zero h tiles to fill the t=1->t=2 wavefront bubble (308us -- the
   extra stream work exceeded the ~7us bubble it replaced); warmup 28 vs
   16 MMs: noise.

Negative results (all reverted, kept here so they are not retried): every
scheme that delays or coarsens the per-chunk tail chain loses big. The
per-chunk chain [io-sigmoid -> g/f ACT -> c-update DVEs -> tanh(c) ->
h-staging mul -> 4-5 halo DMAs] is ~6.5us and the next step's matmuls
wait on exactly these halo writes ~4 wavefront slots later (~13us runway,
roughly half of it eaten by ACT/DVE/Sync queue contention):
 - weight-outer MM emission over step-local 2-chunk groups with [128,1024]
   PSUM tiles + 1024-wide ACT/DVE tails: 506us (PE stalled ~6us every
   step boundary, HAM re-throttled, throttle_active 220us).
 - weight-outer pairing of wavefront-adjacent (step t, step t+1) chunks:
   488us (the fresh-dependency member stalls its whole pair).
 - intra-step adjacent pairing with per-chunk tails: 404us.
 - batching tanh(c)+staging+halo DMAs per 2-chunk group (singles MMs):
   387us. LAG=6/7 instead of 5: no effect (queue latency, not emission
   order, is binding).
The ~100ns LDWEIGHTS stall on ~4 MMs/chunk (single background weight
buffer, single-use weights) is the price of the fine-grained pipeline;
all attempts to amortize it cost more than the ~37us it wastes.

fp8 DoubleRow (2x PE) is numerically dead here: bf16 matmul quantization
already accounts for ~8.8e-3 of the 2e-2 rel-err budget and fp8e4m3 is
16x coarser. A pairwise-AllReduce halo exchange (to drop the shrinking
halo recompute, -23% work) measures 6.5-7.5us per CC op + ~5us of
bounce DMAs/sem props -- longer than the ~13us/step of hideable slack,
so it was not pursued.
"""
import sys
import os

for _p in ("/opt/trn_rl_repo", "/root/.axon_site"):
    if _p not in sys.path and os.path.isdir(_p):
        sys.path.append(_p)

import numpy as np

T, B, C, H, W, HID = 16, 4, 32, 64, 64, 64
HP, WP = 49, 66          # padded per-core image: 48 data rows + 1 top pad, 64+2 cols
FLAT = HP * WP           # 3234
NR = 8                   # output rows per chunk (N = NR*64 = 512 <= PSUM bank)
XLEN = 3100              # per-block x DMA length (covers max read f=3099)

MM_DT = os.environ.get("KLSTM_MM_DT", "bf16")   # "bf16" | "f32r"

X_TAPS_A = [(0, 0), (0, 1), (0, 2), (1, 0)]
X_TAPS_B = [(1, 1), (1, 2), (2, 0), (2, 1)]

_CACHE = {}


def _build_program():
    import concourse.mybir as mybir
    import concourse.tile as tile
    from concourse import bacc

    f32 = mybir.dt.float32
    dtm = mybir.dt.bfloat16 if MM_DT == "bf16" else mybir.dt.float32r

    nc = bacc.Bacc("TRN2", target_bir_lowering=False, debug=False, num_devices=8)

    xp_d = nc.dram_tensor("xp", [T, C, FLAT], dtm, kind="ExternalInput")
    wx1_d = nc.dram_tensor("wx1", [128, 256], dtm, kind="ExternalInput")
    wx2_d = nc.dram_tensor("wx2", [128, 256], dtm, kind="ExternalInput")
    wa_d = nc.dram_tensor("wa", [128, 768], dtm, kind="ExternalInput")
    wb2_d = nc.dram_tensor("wb2", [128, 256], dtm, kind="ExternalInput")
    wc_d = nc.dram_tensor("wc", [96, 256], dtm, kind="ExternalInput")
    bias_d = nc.dram_tensor("bias", [128, 2], f32, kind="ExternalInput")
    out_d = nc.dram_tensor("out", [T, HID, 32 * 66], f32, kind="ExternalOutput")

    Sigmoid = mybir.ActivationFunctionType.Sigmoid
    Tanh = mybir.ActivationFunctionType.Tanh

    with tile.TileContext(nc) as tc:
        with tc.tile_pool(name="const", bufs=1) as constp, \
             tc.tile_pool(name="xpool", bufs=4 if MM_DT == "bf16" else 2) as xpool, \
             tc.tile_pool(name="hpool", bufs=1) as hpool, \
             tc.tile_pool(name="cpool", bufs=1) as cpool, \
             tc.tile_pool(name="psum", bufs=4, space="PSUM") as psum, \
             tc.tile_pool(name="ifsp", bufs=6) as ifsp, \
             tc.tile_pool(name="smallp", bufs=6) as smallp:

            wx1_s = constp.tile([128, 256], dtm)
            wx2_s = constp.tile([128, 256], dtm)
            wa_s = constp.tile([128, 768], dtm)
            wb2_s = constp.tile([128, 256], dtm)
            wc_s = constp.tile([96, 256], dtm)
            bias_s = constp.tile([128, 2], f32)
            for s_, d_ in [(wa_s, wa_d), (wx1_s, wx1_d), (wx2_s, wx2_d),
                           (wb2_s, wb2_d), (wc_s, wc_d), (bias_s, bias_d)]:
                nc.sync.dma_start(s_[:], d_[:])

            # HAM warmup: junk matmuls during the startup ramp (while the
            # first x tiles load) so step 1's matmuls run at 2.4 GHz. The
            # operand is a locally-memset dummy (vector queue, ~1us) so the
            # warmup starts at ~3us instead of waiting ~13us for a weight
            # DMA through the sync queue; 16 cold MMs (~7us) land exactly
            # when the step-1 x tiles arrive (~8us).
            dummy = constp.tile([128, 512], dtm)
            nc.vector.memset(dummy[:] if MM_DT == "bf16"
                             else dummy[:].bitcast(f32), 0.0)
            wrm = psum.tile([128, 512], f32, tag="psA", name="warm")
            for _ in range(16):
                nc.tensor.matmul(wrm[:, 0:512], dummy[:, 0:128],
                                 dummy[:, 0:512], start=True, stop=True)

            # ping-pong h tiles (shifted partition blocks, see module docstring)
            hA = [hpool.tile([128, FLAT], dtm, tag=f"hA{i}", name=f"hA{i}")
                  for i in range(2)]
            hB2 = [hpool.tile([128, FLAT], dtm, tag=f"hB2{i}", name=f"hB2{i}")
                   for i in range(2)]
            # cmb: parts 0-63 copy of h tap (0,2) rows, parts 64-95 x tap
            # (2,2) shifted +66; both read at row y0+1 by the 96-row MM.
            # 3-deep so step t+1's x-load never waits on step t-1's matmuls
            cmb = [hpool.tile([96, FLAT], dtm, tag=f"cmb{i}", name=f"cmb{i}")
                   for i in range(3)]
            # (the former 66-pitch hout staging ring is gone: the h mul now
            # writes its padded-row layout directly into hA[0:64])
            # init order matters for the startup ramp: cmb memsets go first
            # (vector queue) because load_x(1) writes into cmb[1]; the x
            # loads (gpsimd) are emitted next -- see below -- so they are
            # NOT stuck behind ~25us of memsets (that startup PE-idle
            # re-throttled HAM and ran step 1 cold); remaining memsets
            # split across vector/gpsimd.
            def _ms(eng, t_):
                eng.memset(
                    t_[:] if MM_DT == "bf16" else t_[:].bitcast(f32), 0.0)
            for t_ in cmb:
                _ms(nc.vector, t_)

            c_s = cpool.tile([64, 47 * 64], f32)

            def load_x(t):
                # x tiles for step t: TA (4 blocks), TB (4 blocks), cmb x-block
                # (issued from the mostly-idle gpsimd queue, ~one step ahead)
                xa = xpool.tile([128, FLAT], dtm, tag="xa", name="xa")
                xb = xpool.tile([128, FLAT], dtm, tag="xb", name="xb")
                for b3, (dy, dx) in enumerate(X_TAPS_A):
                    s = dy * WP + dx
                    nc.gpsimd.dma_start(xa[32 * b3:32 * b3 + 32, 0:XLEN],
                                        xp_d[t - 1, :, s:s + XLEN])
                for b3, (dy, dx) in enumerate(X_TAPS_B):
                    s = dy * WP + dx
                    nc.gpsimd.dma_start(xb[32 * b3:32 * b3 + 32, 0:XLEN],
                                        xp_d[t - 1, :, s:s + XLEN])
                nc.gpsimd.dma_start(cmb[t % 3][64:96, 66:66 + XLEN],
                                    xp_d[t - 1, :, 134:134 + XLEN])
                return xa, xb

            def rv(tile_ap):
                return tile_ap.rearrange("p (y x) -> p y x", x=WP)

            xload = {1: load_x(1)}
            for t_ in hA + hB2:
                _ms(nc.vector, t_)
            ctx = {}

            def step_ctx(t):
                if t not in ctx:
                    if t + 1 <= T and t + 1 not in xload:
                        xload[t + 1] = load_x(t + 1)
                    xa, xb = xload[t]
                    ctx[t] = dict(
                        xav=rv(xa[:]), xbv=rv(xb[:]),
                        hAv=rv(hA[(t - 1) % 2][:]),
                        hB2v=rv(hB2[(t - 1) % 2][:]),
                        cmbv=rv(cmb[t % 3][:]),
                        hAc=hA[t % 2], hB2c=hB2[t % 2],
                        cmbn=cmb[(t + 1) % 3],
                    )
                return ctx[t]

            def weight_seq(t, cx, h):
                # the 7 (or 3 at t=1) stationary weights of one output-half,
                # each with its rhs builder
                hs = h * 128
                seq = [
                    (wx1_s[:, hs:hs + 128],
                     lambda y0, nr: cx["xav"][:, y0:y0 + nr, 0:64]),
                    (wx2_s[:, hs:hs + 128],
                     lambda y0, nr: cx["xbv"][:, y0:y0 + nr, 0:64]),
                ]
                if t > 1:
                    seq += [(wa_s[:, (dy * 2 + h) * 128:(dy * 2 + h + 1) * 128],
                             lambda y0, nr, dy=dy:
                             cx["hAv"][:, y0 + dy:y0 + dy + nr, 0:64])
                            for dy in range(3)]
                    seq += [(wb2_s[:, hs:hs + 128],
                             lambda y0, nr: cx["hB2v"][:, y0:y0 + nr, 0:64])]
                seq += [(wc_s[0:96, hs:hs + 128],
                         lambda y0, nr:
                         cx["cmbv"][0:96, y0 + 1:y0 + 1 + nr, 0:64])]
                return seq

            def front_pair(pair):
                # weight-outer emission over a pair of wavefront-adjacent
                # chunks (same weights for all t>=2): each stationary weight
                # issues 2 back-to-back MMs so the next LDWEIGHTS always has
                # a full MM-stream shadow to hide in.
                for job in pair:
                    job["pa"] = psum.tile([128, 512], f32, tag="psA",
                                          name="psA")
                    job["pb"] = psum.tile([128, 512], f32, tag="psB",
                                          name="psB")
                    job["cx"] = step_ctx(job["t"])
                # pb's half ([g; f]) is emitted FIRST so the tail's gt/fs
                # activations and the f*c mul run concurrently with pa's
                # matmuls, cutting the post-chunk halo chain from ~5 to
                # ~3.3us
                for h in (1, 0):
                    seqs = [weight_seq(job["t"], job["cx"], h) for job in pair]
                    nw = max(len(s) for s in seqs)
                    for wi in range(nw):
                        for job, seq in zip(pair, seqs):
                            if wi >= len(seq):
                                continue
                            lhsT, rhs_of = seq[wi]
                            y0, nr = job["y0"], job["nr"]
                            pt = job["pa"] if h == 0 else job["pb"]
                            nc.tensor.matmul(
                                pt[:, :nr * 64], lhsT, rhs_of(y0, nr),
                                start=(wi == 0), stop=(wi == len(seq) - 1))

            def tail_chunk(job):
                # per-chunk gate activations + c update (short dependency
                # chain, fine-grained pipelining)
                t, y0, nr = job["t"], job["y0"], job["nr"]
                pa, pb = job["pa"], job["pb"]
                N = nr * 64
                # pa = [i; o] -> one merged sigmoid; pb = [g; f]. pb ops
                # come first (its matmuls finish mid-chunk): gt/fs and the
                # f*c mul overlap pa's matmul half.
                gt = smallp.tile([64, 512], f32, tag="gt")
                nc.scalar.activation(gt[:, :N], pb[0:64, :N], Tanh,
                                     bias=bias_s[0:64, 1:2])
                c_sl = c_s[:, y0 * 64:y0 * 64 + N]
                io = ifsp.tile([128, 512], f32, tag="io", name="io")
                if t == 1:
                    nc.scalar.activation(io[:, :N], pa[0:128, :N], Sigmoid,
                                         bias=bias_s[0:128, 0:1])
                    nc.vector.tensor_mul(c_sl, io[0:64, :N], gt[:, :N])
                else:
                    fs_ = ifsp.tile([64, 512], f32, tag="fs", name="fs_")
                    nc.scalar.activation(fs_[:, :N], pb[64:128, :N],
                                         Sigmoid, bias=bias_s[64:128, 1:2])
                    nc.vector.tensor_mul(c_sl, fs_[:, :N], c_sl)
                    nc.scalar.activation(io[:, :N], pa[0:128, :N], Sigmoid,
                                         bias=bias_s[0:128, 0:1])
                    t1 = smallp.tile([64, 512], f32, tag="t1")
                    nc.vector.tensor_mul(t1[:, :N], io[0:64, :N], gt[:, :N])
                    nc.vector.tensor_add(c_sl, c_sl, t1[:, :N])
                # tanh(c) cross-written to parts 64-127, next to o
                tc_ = smallp.tile([128, 512], f32, tag="tc")
                nc.scalar.activation(tc_[64:128, :N], c_sl, Tanh)
                cx = job["cx"]
                L = nr * 66
                d0 = (y0 + 1) * 66
                hAc, hB2c, cmbn = cx["hAc"], cx["hB2c"], cx["cmbn"]
                if t < T or y0 < 32:
                    # the h staging mul writes DIRECTLY into hA[0:64] (its
                    # layout -- row r at 66-pitch, cols 1:65 -- IS the
                    # tap-(dy,0) halo image, W-pad cols stay memset-0):
                    # the most latency-critical halo write needs no DMA at
                    # all, and the other shifted copies source from here.
                    hA3 = hAc[0:64].rearrange("p (y x) -> p y x", x=66)
                    o3 = io[64:128, :N].rearrange("p (y x) -> p y x", x=64)
                    t3 = tc_[64:128, :N].rearrange("p (y x) -> p y x", x=64)
                    nc.vector.tensor_mul(
                        hA3[:, y0 + 1:y0 + 1 + nr, 1:65], o3, t3)
                if t < T:
                    # whole-row contiguous shifted copies out of hA[0:64]
                    # (1 desc/partition); trailing stale elements land only
                    # in never-read pad columns (>=64) of the destinations.
                    # The cmb copy (for the merged 96-row MM) rides the
                    # lighter gpsimd queue.
                    # +2-shift transfers are trimmed 2 elements short so the
                    # source never touches the next chunk's row region (its
                    # mul writes col 1 there -- a would-be serialization);
                    # the dropped destination bytes are pad cols 64/65.
                    hsrc = hAc[0:64]
                    nc.sync.dma_start(hAc[64:128, d0:d0 + L],
                                      hsrc[:, d0 + 1:d0 + 1 + L])
                    nc.sync.dma_start(hB2c[0:64, d0:d0 + L - 2],
                                      hsrc[:, d0 + 2:d0 + L])
                    nc.gpsimd.dma_start(cmbn[0:64, d0:d0 + L - 2],
                                        hsrc[:, d0 + 2:d0 + L])
                    if y0 == 0:
                        nc.sync.dma_start(hB2c[64:128, 0:L - 68],
                                          hsrc[:, d0 + 68:d0 + L])
                    else:
                        d4 = (y0 - 1) * 66
                        nc.sync.dma_start(hB2c[64:128, d4:d4 + L - 2],
                                          hsrc[:, d0 + 2:d0 + L])
                if y0 < 32:
                    # out rows via casting SWDGE DMA straight from the bf16
                    # halo rows (padded 66-el layout; host strips)
                    src = (hAc[0:64, d0:d0 + L] if MM_DT == "bf16"
                           else hAc[0:64, d0:d0 + L].bitcast(f32))
                    nc.gpsimd.dma_start(
                        out_d[t - 1, :, y0 * 66:y0 * 66 + L], src)

            # cross-step wavefront: step t+1's chunk q is emitted alongside
            # step t's chunk q+LAG, so the next step's matmuls depend only on
            # halo writes that are already emitted (their true inputs) and
            # the PE never stalls at a step boundary. MM emission is fused
            # over INTRA-step adjacent chunk pairs (at the even chunk's
            # wavefront slot): both members' inputs are >=4 slots mature, so
            # the pair never waits (pairing a step-t chunk with a step-t+1
            # chunk here stalled the PE ~5us per pair on fresh halo writes).
            LAG = int(os.environ.get("KLSTM_LAG", "5"))
            pairs = []
            for t in range(1, T + 1):
                R = 48 - t
                nch = (R + NR - 1) // NR
                chunks = [dict(t=t, q=q, y0=q * NR, nr=min(NR, R - q * NR))
                          for q in range(nch)]
                PW = int(os.environ.get("KLSTM_PAIR", "1"))
                for k in range(0, nch, PW):
                    pr = chunks[k:k + PW]
                    pairs.append((LAG * (t - 1) + k, t, pr))
            pairs.sort(key=lambda p: (p[0], p[1]))
            for _, t, pr in pairs:
                front_pair(pr)
                for job in pr:
                    tail_chunk(job)
    nc.compile()
    return nc


def _host_prep(x, w_x2h, b_x2h, w_h2h, b_h2h):
    """Build the 8 per-core input maps."""
    import ml_dtypes
    np_dtm = ml_dtypes.bfloat16 if MM_DT == "bf16" else np.float32

    x = np.ascontiguousarray(np.asarray(x, np.float32))
    w_x2h = np.asarray(w_x2h, np.float32)
    b_x2h = np.asarray(b_x2h, np.float32)
    w_h2h = np.asarray(w_h2h, np.float32)
    b_h2h = np.asarray(b_h2h, np.float32)

    # gate-channel permutation: [i, o, g, f] so psum half0=[i;o], half1=[g;f]
    order = np.r_[0:64, 192:256, 128:192, 64:128]

    bias = np.zeros((128, 2), np.float32)
    bsum = (b_x2h + b_h2h)[order]
    bias[:, 0] = bsum[0:128]
    bias[:, 1] = bsum[128:256]

    in_maps = []
    packed_w = {}
    for parity in range(2):
        wx_f = (w_x2h if parity == 0 else w_x2h[:, :, ::-1, :])[order]
        wh_f = (w_h2h if parity == 0 else w_h2h[:, :, ::-1, :])[order]
        wx1 = np.zeros((128, 2, 128), np.float32)
        wx2 = np.zeros((128, 2, 128), np.float32)
        wa = np.zeros((128, 3, 2, 128), np.float32)
        wb2 = np.zeros((128, 2, 128), np.float32)
        wc = np.zeros((96, 2, 128), np.float32)
        for hh in range(2):
            oc = slice(hh * 128, (hh + 1) * 128)
            for b3, (dy, dx) in enumerate(X_TAPS_A):
                wx1[32 * b3:32 * b3 + 32, hh, :] = wx_f[oc, :, dy, dx].T
            for b3, (dy, dx) in enumerate(X_TAPS_B):
                wx2[32 * b3:32 * b3 + 32, hh, :] = wx_f[oc, :, dy, dx].T
            for dy in range(3):
                for b3 in range(2):
                    wa[64 * b3:64 * b3 + 64, dy, hh, :] = wh_f[oc, :, dy, b3].T
            wb2[0:64, hh, :] = wh_f[oc, :, 0, 2].T
            wb2[64:128, hh, :] = wh_f[oc, :, 2, 2].T
            wc[0:64, hh, :] = wh_f[oc, :, 1, 2].T
            wc[64:96, hh, :] = wx_f[oc, :, 2, 2].T
        packed_w[parity] = tuple(
            np.ascontiguousarray(a.reshape(a.shape[0], -1).astype(np_dtm))
            for a in (wx1, wx2, wa, wb2, wc))

    for core in range(8):
        b, parity = core // 2, core % 2
        xv = x[:, b]
        if parity == 1:
            xv = xv[:, :, ::-1, :]
        xp = np.zeros((T, C, HP, WP), np.float32)
        xp[:, :, 1:49, 1:65] = xv[:, :, 0:48, :]
        wx1, wx2, wa, wb2, wc = packed_w[parity]
        in_maps.append({
            "xp": np.ascontiguousarray(xp.reshape(T, C, FLAT).astype(np_dtm)),
            "wx1": wx1, "wx2": wx2, "wa": wa, "wb2": wb2, "wc": wc,
            "bias": bias,
        })
    return in_maps


def kernel(x, w_x2h, b_x2h, w_h2h, b_h2h, _trace=False, _tmpdir=None):
    from concourse.bass_utils import run_bass_kernel_spmd

    if "nc" not in _CACHE:
        _CACHE["nc"] = _build_program()
    nc = _CACHE["nc"]

    in_maps = _host_prep(x, w_x2h, b_x2h, w_h2h, b_h2h)
    kw = {}
    if _trace:
        kw = dict(trace=True, tmpdir=_tmpdir)
    res = run_bass_kernel_spmd(nc, in_maps, core_ids=list(range(8)), **kw)

    full = np.zeros((T, B, HID, H, W), np.float32)
    for core in range(8):
        b, parity = core // 2, core % 2
        out = res.results[core]["out"].reshape(T, HID, 32, 66)[:, :, :, 1:65]
        if parity == 0:
            full[:, b, :, 0:32] = out
        else:
            full[:, b, :, 32:64] = out[:, :, ::-1, :]
    if _trace:
        return full, res
    return full


# revision 49
# speedup vs baseline: 1.0389x; 1.0057x over previous
"""ConvLSTM (T=16, B=4, C=32, HID=64, 64x64, 3x3 convs) on 8 Trainium2 cores.

Decomposition: 8 cores = batch(4) x H-halves(2). Each core owns 32 output rows
and recomputes a shrinking halo (rows 0..47-t at step t) so NO inter-core
communication is needed. The bottom-half cores get vertically flipped inputs
(and dy-flipped conv weights) so all 8 cores run the identical SPMD program.

Conv-as-matmul with dense tap packing: the 9 taps x 32ch (x2h) and 9 taps x
64ch (h2h) of the two 3x3 convs are packed into the 128-partition contraction
dim as shifted copies of the padded image, reaching the floor of 7 matmuls
per 128-out-channel half per 512-pixel chunk:
    X1 = x taps (0,0)(0,1)(0,2)(1,0)   [4x32 rows, shifts 0,1,2,66]
    X2 = x taps (1,1)(1,2)(2,0)(2,1)   [shifts 67,68,132,133]
    HA @ dy*66, dy=0,1,2 = h taps (dy,0)(dy,1)   [2x64 rows, shifts 0,1]
    HB2 = h taps (0,2)(2,2)            [shifts 2,134]
    C96 = h tap (1,2) + x tap (2,2)    [96 rows; cmb holds a copy of tap
          (0,2) on parts 0:64 and the x block shifted +66 on parts 64:96,
          both read at row y0+1]
x blocks are DMA'd from DRAM at shifted offsets (free); h blocks are written
by 5 whole-row SBUF DMAs per 2-chunk group. All matmuls accumulate at base
partition 0 (mixed-base accumulation hangs the HW). MM_DT selects matmul
precision: bf16 (fast) or float32r (~20x more precise, ~1.7x slower PE).

Perf round 1 (479us -> 398us): padded 66-pitch staging ring for whole-row
halo DMAs (1 descriptor/partition); [i;o]/[g;f] gate packing for merged
sigmoid + ACT cross-partition-base writes.

Perf round 2 (398us -> ~326us):
 - the 64-row h-tap-(1,2) and 32-row x-tap-(2,2) matmuls merge into one
   96-row MM (14 MMs/chunk, the contraction floor of ceil(864/128) x 2):
   cmb parts 0:64 get a copy of the tap-(0,2) halo rows (extra per-chunk
   DMA on the lightly-loaded gpsimd queue) and the cmb x DMA lands
   shifted +66 so both halves read at row y0+1.
 - startup ramp fix: the step-1 x loads used to queue behind ~25us of
   init memsets on gpsimd, leaving a ~19us PE-idle that re-throttled HAM
   (PE clock gate) back to 1.2 GHz for step 1. Now cmb memsets go first
   on the vector queue, the x loads are emitted immediately after on
   gpsimd, remaining memsets split vector/gpsimd, and the 16-MM warmup
   reads a locally-memset dummy tile so it starts at ~3us instead of
   waiting ~13us for a weight DMA (first real MM ~15us, was ~25us).
 - routing the i*g mul to the gpsimd Pool engine or halo DMAs onto a
   6-deep sync queue regressed to 513us (Pool tensor ops are far slower
   on HW than the cost model's 0.42 efficiency suggests); bf16 gate
   tiles (for the DVE all-2-byte 2x mode) gained nothing and cost
   precision (1.0e-2) -- both reverted.

Perf round 3 (~326us -> ~302us):
 - the h staging mul (o * tanh(c)) writes DIRECTLY into hA[0:64]: its
   66-pitch padded-row layout IS the tap-(dy,0) halo image, so the most
   latency-critical halo write needs no DMA at all (saves ~1.9us of
   issue+transfer+sem-prop on the cross-step chain), the other shifted
   copies source from hA[0:64], the out DMA reads it too, and the whole
   staging ring disappears. The +2-shift copies are trimmed 2 elements so
   they never read the next chunk's row region (its mul writes col 1
   there -- Tile would serialize the next mul behind the copy); dropped
   destination bytes are only ever pad cols >=64, which no matmul reads.
 - also tried and reverted: emitting full 14-MM blocks at t=1 against the
   memset-zero h tiles to fill the t=1->t=2 wavefront bubble (308us -- the
   extra stream work exceeded the ~7us bubble it replaced); warmup 28 vs
   16 MMs: noise.

Negative results (all reverted, kept here so they are not retried): every
scheme that delays or coarsens the per-chunk tail chain loses big. The
per-chunk chain [io-sigmoid -> g/f ACT -> c-update DVEs -> tanh(c) ->
h-staging mul -> 4-5 halo DMAs] is ~6.5us and the next step's matmuls
wait on exactly these halo writes ~4 wavefront slots later (~13us runway,
roughly half of it eaten by ACT/DVE/Sync queue contention):
 - weight-outer MM emission over step-local 2-chunk groups with [128,1024]
   PSUM tiles + 1024-wide ACT/DVE tails: 506us (PE stalled ~6us every
   step boundary, HAM re-throttled, throttle_active 220us).
 - weight-outer pairing of wavefront-adjacent (step t, step t+1) chunks:
   488us (the fresh-dependency member stalls its whole pair).
 - intra-step adjacent pairing with per-chunk tails: 404us.
 - batching tanh(c)+staging+halo DMAs per 2-chunk group (singles MMs):
   387us. LAG=6/7 instead of 5: no effect (queue latency, not emission
   order, is binding).
The ~100ns LDWEIGHTS stall on ~4 MMs/chunk (single background weight
buffer, single-use weights) is the price of the fine-grained pipeline;
all attempts to amortize it cost more than the ~37us it wastes.

fp8 DoubleRow (2x PE) is numerically dead here: bf16 matmul quantization
already accounts for ~8.8e-3 of the 2e-2 rel-err budget and fp8e4m3 is
16x coarser. A pairwise-AllReduce halo exchange (to drop the shrinking
halo recompute, -23% work) measures 6.5-7.5us per CC op + ~5us of
bounce DMAs/sem props -- longer than the ~13us/step of hideable slack,
so it was not pursued.
"""
import sys
import os

for _p in ("/opt/trn_rl_repo", "/root/.axon_site"):
    if _p not in sys.path and os.path.isdir(_p):
        sys.path.append(_p)

import numpy as np

T, B, C, H, W, HID = 16, 4, 32, 64, 64, 64
HP, WP = 49, 66          # padded per-core image: 48 data rows + 1 top pad, 64+2 cols
FLAT = HP * WP           # 3234
NR = 8                   # output rows per chunk (N = NR*64 = 512 <= PSUM bank)
XLEN = 3100              # per-block x DMA length (covers max read f=3099)

MM_DT = os.environ.get("KLSTM_MM_DT", "bf16")   # "bf16" | "f32r"

X_TAPS_A = [(0, 0), (0, 1), (0, 2), (1, 0)]
X_TAPS_B = [(1, 1), (1, 2), (2, 0), (2, 1)]

_CACHE = {}


def _build_program():
    import concourse.mybir as mybir
    import concourse.tile as tile
    from concourse import bacc

    f32 = mybir.dt.float32
    dtm = mybir.dt.bfloat16 if MM_DT == "bf16" else mybir.dt.float32r

    nc = bacc.Bacc("TRN2", target_bir_lowering=False, debug=False, num_devices=8)

    xp_d = nc.dram_tensor("xp", [T, C, FLAT], dtm, kind="ExternalInput")
    wx1_d = nc.dram_tensor("wx1", [128, 256], dtm, kind="ExternalInput")
    wx2_d = nc.dram_tensor("wx2", [128, 256], dtm, kind="ExternalInput")
    wa_d = nc.dram_tensor("wa", [128, 768], dtm, kind="ExternalInput")
    wb2_d = nc.dram_tensor("wb2", [128, 256], dtm, kind="ExternalInput")
    wc_d = nc.dram_tensor("wc", [96, 256], dtm, kind="ExternalInput")
    bias_d = nc.dram_tensor("bias", [128, 2], f32, kind="ExternalInput")
    out_d = nc.dram_tensor("out", [T, HID, 32 * 66], f32, kind="ExternalOutput")

    Sigmoid = mybir.ActivationFunctionType.Sigmoid
    Tanh = mybir.ActivationFunctionType.Tanh

    with tile.TileContext(nc) as tc:
        with tc.tile_pool(name="const", bufs=1) as constp, \
             tc.tile_pool(name="xpool", bufs=4 if MM_DT == "bf16" else 2) as xpool, \
             tc.tile_pool(name="hpool", bufs=1) as hpool, \
             tc.tile_pool(name="cpool", bufs=1) as cpool, \
             tc.tile_pool(name="psum", bufs=4, space="PSUM") as psum, \
             tc.tile_pool(name="ifsp", bufs=6) as ifsp, \
             tc.tile_pool(name="smallp", bufs=6) as smallp:

            wx1_s = constp.tile([128, 256], dtm)
            wx2_s = constp.tile([128, 256], dtm)
            wa_s = constp.tile([128, 768], dtm)
            wb2_s = constp.tile([128, 256], dtm)
            wc_s = constp.tile([96, 256], dtm)
            bias_s = constp.tile([128, 2], f32)
            for s_, d_ in [(wa_s, wa_d), (wx1_s, wx1_d), (wx2_s, wx2_d),
                           (wb2_s, wb2_d), (wc_s, wc_d), (bias_s, bias_d)]:
                nc.sync.dma_start(s_[:], d_[:])

            # HAM warmup: junk matmuls during the startup ramp (while the
            # first x tiles load) so step 1's matmuls run at 2.4 GHz. The
            # operand is a locally-memset dummy (vector queue, ~1us) so the
            # warmup starts at ~3us instead of waiting ~13us for a weight
            # DMA through the sync queue; 16 cold MMs (~7us) land exactly
            # when the step-1 x tiles arrive (~8us).
            dummy = constp.tile([128, 512], dtm)
            nc.vector.memset(dummy[:] if MM_DT == "bf16"
                             else dummy[:].bitcast(f32), 0.0)
            wrm = psum.tile([128, 512], f32, tag="psA", name="warm")
            for _ in range(16):
                nc.tensor.matmul(wrm[:, 0:512], dummy[:, 0:128],
                                 dummy[:, 0:512], start=True, stop=True)

            # ping-pong h tiles (shifted partition blocks, see module docstring)
            hA = [hpool.tile([128, FLAT], dtm, tag=f"hA{i}", name=f"hA{i}")
                  for i in range(2)]
            hB2 = [hpool.tile([128, FLAT], dtm, tag=f"hB2{i}", name=f"hB2{i}")
                   for i in range(2)]
            # cmb: parts 0-63 copy of h tap (0,2) rows, parts 64-95 x tap
            # (2,2) shifted +66; both read at row y0+1 by the 96-row MM.
            # 3-deep so step t+1's x-load never waits on step t-1's matmuls
            cmb = [hpool.tile([96, FLAT], dtm, tag=f"cmb{i}", name=f"cmb{i}")
                   for i in range(3)]
            # (the former 66-pitch hout staging ring is gone: the h mul now
            # writes its padded-row layout directly into hA[0:64])
            # init order matters for the startup ramp: cmb memsets go first
            # (vector queue) because load_x(1) writes into cmb[1]; the x
            # loads (gpsimd) are emitted next -- see below -- so they are
            # NOT stuck behind ~25us of memsets (that startup PE-idle
            # re-throttled HAM and ran step 1 cold); remaining memsets
            # split across vector/gpsimd.
            def _ms(eng, t_):
                eng.memset(
                    t_[:] if MM_DT == "bf16" else t_[:].bitcast(f32), 0.0)
            for t_ in cmb:
                _ms(nc.vector, t_)

            c_s = cpool.tile([64, 47 * 64], f32)

            def load_x(t):
                # x tiles for step t: TA (4 blocks), TB (4 blocks), cmb x-block
                # (issued from the mostly-idle gpsimd queue, ~one step ahead).
                # Step 1's loads are split into a prefix covering chunks 0-1
                # (flat <= 1400) plus the remainder, so the very first
                # matmuls unblock ~2x earlier during the startup ramp; later
                # steps' loads hide under a full step and stay whole.
                xa = xpool.tile([128, FLAT], dtm, tag="xa", name="xa")
                xb = xpool.tile([128, FLAT], dtm, tag="xb", name="xb")
                cuts = [0, 1400, XLEN] if t == 1 else [0, XLEN]
                for c0, c1 in zip(cuts, cuts[1:]):
                    for b3, (dy, dx) in enumerate(X_TAPS_A):
                        s = dy * WP + dx
                        nc.gpsimd.dma_start(
                            xa[32 * b3:32 * b3 + 32, c0:c1],
                            xp_d[t - 1, :, s + c0:s + c1])
                    for b3, (dy, dx) in enumerate(X_TAPS_B):
                        s = dy * WP + dx
                        nc.gpsimd.dma_start(
                            xb[32 * b3:32 * b3 + 32, c0:c1],
                            xp_d[t - 1, :, s + c0:s + c1])
                    nc.gpsimd.dma_start(
                        cmb[t % 3][64:96, 66 + c0:66 + c1],
                        xp_d[t - 1, :, 134 + c0:134 + c1])
                return xa, xb

            def rv(tile_ap):
                return tile_ap.rearrange("p (y x) -> p y x", x=WP)

            xload = {1: load_x(1)}
            for t_ in hA + hB2:
                _ms(nc.vector, t_)
            ctx = {}

            def step_ctx(t):
                if t not in ctx:
                    if t + 1 <= T and t + 1 not in xload:
                        xload[t + 1] = load_x(t + 1)
                    xa, xb = xload[t]
                    ctx[t] = dict(
                        xav=rv(xa[:]), xbv=rv(xb[:]),
                        hAv=rv(hA[(t - 1) % 2][:]),
                        hB2v=rv(hB2[(t - 1) % 2][:]),
                        cmbv=rv(cmb[t % 3][:]),
                        hAc=hA[t % 2], hB2c=hB2[t % 2],
                        cmbn=cmb[(t + 1) % 3],
                    )
                return ctx[t]

            def weight_seq(t, cx, h):
                # the 7 (or 3 at t=1) stationary weights of one output-half,
                # each with its rhs builder
                hs = h * 128
                seq = [
                    (wx1_s[:, hs:hs + 128],
                     lambda y0, nr: cx["xav"][:, y0:y0 + nr, 0:64]),
                    (wx2_s[:, hs:hs + 128],
                     lambda y0, nr: cx["xbv"][:, y0:y0 + nr, 0:64]),
                ]
                if t > 1:
                    seq += [(wa_s[:, (dy * 2 + h) * 128:(dy * 2 + h + 1) * 128],
                             lambda y0, nr, dy=dy:
                             cx["hAv"][:, y0 + dy:y0 + dy + nr, 0:64])
                            for dy in range(3)]
                    seq += [(wb2_s[:, hs:hs + 128],
                             lambda y0, nr: cx["hB2v"][:, y0:y0 + nr, 0:64])]
                seq += [(wc_s[0:96, hs:hs + 128],
                         lambda y0, nr:
                         cx["cmbv"][0:96, y0 + 1:y0 + 1 + nr, 0:64])]
                return seq

            def front_pair(pair):
                # weight-outer emission over a pair of wavefront-adjacent
                # chunks (same weights for all t>=2): each stationary weight
                # issues 2 back-to-back MMs so the next LDWEIGHTS always has
                # a full MM-stream shadow to hide in.
                for job in pair:
                    job["pa"] = psum.tile([128, 512], f32, tag="psA",
                                          name="psA")
                    job["pb"] = psum.tile([128, 512], f32, tag="psB",
                                          name="psB")
                    job["cx"] = step_ctx(job["t"])
                # pb's half ([g; f]) is emitted FIRST so the tail's gt/fs
                # activations and the f*c mul run concurrently with pa's
                # matmuls, cutting the post-chunk halo chain from ~5 to
                # ~3.3us
                for h in (1, 0):
                    seqs = [weight_seq(job["t"], job["cx"], h) for job in pair]
                    nw = max(len(s) for s in seqs)
                    for wi in range(nw):
                        for job, seq in zip(pair, seqs):
                            if wi >= len(seq):
                                continue
                            lhsT, rhs_of = seq[wi]
                            y0, nr = job["y0"], job["nr"]
                            pt = job["pa"] if h == 0 else job["pb"]
                            nc.tensor.matmul(
                                pt[:, :nr * 64], lhsT, rhs_of(y0, nr),
                                start=(wi == 0), stop=(wi == len(seq) - 1))

            def tail_chunk(job):
                # per-chunk gate activations + c update (short dependency
                # chain, fine-grained pipelining)
                t, y0, nr = job["t"], job["y0"], job["nr"]
                pa, pb = job["pa"], job["pb"]
                N = nr * 64
                # pa = [i; o] -> one merged sigmoid; pb = [g; f]. pb ops
                # come first (its matmuls finish mid-chunk): gt/fs and the
                # f*c mul overlap pa's matmul half.
                gt = smallp.tile([64, 512], f32, tag="gt")
                nc.scalar.activation(gt[:, :N], pb[0:64, :N], Tanh,
                                     bias=bias_s[0:64, 1:2])
                c_sl = c_s[:, y0 * 64:y0 * 64 + N]
                io = ifsp.tile([128, 512], f32, tag="io", name="io")
                if t == 1:
                    nc.scalar.activation(io[:, :N], pa[0:128, :N], Sigmoid,
                                         bias=bias_s[0:128, 0:1])
                    nc.vector.tensor_mul(c_sl, io[0:64, :N], gt[:, :N])
                else:
                    fs_ = ifsp.tile([64, 512], f32, tag="fs", name="fs_")
                    nc.scalar.activation(fs_[:, :N], pb[64:128, :N],
                                         Sigmoid, bias=bias_s[64:128, 1:2])
                    nc.vector.tensor_mul(c_sl, fs_[:, :N], c_sl)
                    nc.scalar.activation(io[:, :N], pa[0:128, :N], Sigmoid,
                                         bias=bias_s[0:128, 0:1])
                    t1 = smallp.tile([64, 512], f32, tag="t1")
                    nc.vector.tensor_mul(t1[:, :N], io[0:64, :N], gt[:, :N])
                    nc.vector.tensor_add(c_sl, c_sl, t1[:, :N])
                # tanh(c) cross-written to parts 64-127, next to o
                tc_ = smallp.tile([128, 512], f32, tag="tc")
                nc.scalar.activation(tc_[64:128, :N], c_sl, Tanh)
                cx = job["cx"]
                L = nr * 66
                d0 = (y0 + 1) * 66
                hAc, hB2c, cmbn = cx["hAc"], cx["hB2c"], cx["cmbn"]
                if t < T or y0 < 32:
                    # the h staging mul writes DIRECTLY into hA[0:64] (its
                    # layout -- row r at 66-pitch, cols 1:65 -- IS the
                    # tap-(dy,0) halo image, W-pad cols stay memset-0):
                    # the most latency-critical halo write needs no DMA at
                    # all, and the other shifted copies source from here.
                    hA3 = hAc[0:64].rearrange("p (y x) -> p y x", x=66)
                    o3 = io[64:128, :N].rearrange("p (y x) -> p y x", x=64)
                    t3 = tc_[64:128, :N].rearrange("p (y x) -> p y x", x=64)
                    nc.vector.tensor_mul(
                        hA3[:, y0 + 1:y0 + 1 + nr, 1:65], o3, t3)
                if t < T:
                    # whole-row contiguous shifted copies out of hA[0:64]
                    # (1 desc/partition); trailing stale elements land only
                    # in never-read pad columns (>=64) of the destinations.
                    # The cmb copy (for the merged 96-row MM) rides the
                    # lighter gpsimd queue.
                    # +2-shift transfers are trimmed 2 elements short so the
                    # source never touches the next chunk's row region (its
                    # mul writes col 1 there -- a would-be serialization);
                    # the dropped destination bytes are pad cols 64/65.
                    hsrc = hAc[0:64]
                    nc.sync.dma_start(hAc[64:128, d0:d0 + L],
                                      hsrc[:, d0 + 1:d0 + 1 + L])
                    nc.sync.dma_start(hB2c[0:64, d0:d0 + L - 2],
                                      hsrc[:, d0 + 2:d0 + L])
                    nc.gpsimd.dma_start(cmbn[0:64, d0:d0 + L - 2],
                                        hsrc[:, d0 + 2:d0 + L])
                    if y0 == 0:
                        nc.sync.dma_start(hB2c[64:128, 0:L - 68],
                                          hsrc[:, d0 + 68:d0 + L])
                    else:
                        d4 = (y0 - 1) * 66
                        nc.sync.dma_start(hB2c[64:128, d4:d4 + L - 2],
                                          hsrc[:, d0 + 2:d0 + L])
                if y0 < 32:
                    # out rows via casting SWDGE DMA straight from the bf16
                    # halo rows (padded 66-el layout; host strips)
                    src = (hAc[0:64, d0:d0 + L] if MM_DT == "bf16"
                           else hAc[0:64, d0:d0 + L].bitcast(f32))
                    nc.gpsimd.dma_start(
                        out_d[t - 1, :, y0 * 66:y0 * 66 + L], src)

            # cross-step wavefront: step t+1's chunk q is emitted alongside
            # step t's chunk q+LAG, so the next step's matmuls depend only on
            # halo writes that are already emitted (their true inputs) and
            # the PE never stalls at a step boundary. MM emission is fused
            # over INTRA-step adjacent chunk pairs (at the even chunk's
            # wavefront slot): both members' inputs are >=4 slots mature, so
            # the pair never waits (pairing a step-t chunk with a step-t+1
            # chunk here stalled the PE ~5us per pair on fresh halo writes).
            LAG = int(os.environ.get("KLSTM_LAG", "5"))
            pairs = []
            for t in range(1, T + 1):
                R = 48 - t
                nch = (R + NR - 1) // NR
                chunks = [dict(t=t, q=q, y0=q * NR, nr=min(NR, R - q * NR))
                          for q in range(nch)]
                PW = int(os.environ.get("KLSTM_PAIR", "1"))
                for k in range(0, nch, PW):
                    pr = chunks[k:k + PW]
                    pairs.append((LAG * (t - 1) + k, t, pr))
            pairs.sort(key=lambda p: (p[0], p[1]))
            for _, t, pr in pairs:
                front_pair(pr)
                for job in pr:
                    tail_chunk(job)
    nc.compile()
    return nc


def _host_prep(x, w_x2h, b_x2h, w_h2h, b_h2h):
    """Build the 8 per-core input maps."""
    import ml_dtypes
    np_dtm = ml_dtypes.bfloat16 if MM_DT == "bf16" else np.float32

    x = np.ascontiguousarray(np.asarray(x, np.float32))
    w_x2h = np.asarray(w_x2h, np.float32)
    b_x2h = np.asarray(b_x2h, np.float32)
    w_h2h = np.asarray(w_h2h, np.float32)
    b_h2h = np.asarray(b_h2h, np.float32)

    # gate-channel permutation: [i, o, g, f] so psum half0=[i;o], half1=[g;f]
    order = np.r_[0:64, 192:256, 128:192, 64:128]

    bias = np.zeros((128, 2), np.float32)
    bsum = (b_x2h + b_h2h)[order]
    bias[:, 0] = bsum[0:128]
    bias[:, 1] = bsum[128:256]

    in_maps = []
    packed_w = {}
    for parity in range(2):
        wx_f = (w_x2h if parity == 0 else w_x2h[:, :, ::-1, :])[order]
        wh_f = (w_h2h if parity == 0 else w_h2h[:, :, ::-1, :])[order]
        wx1 = np.zeros((128, 2, 128), np.float32)
        wx2 = np.zeros((128, 2, 128), np.float32)
        wa = np.zeros((128, 3, 2, 128), np.float32)
        wb2 = np.zeros((128, 2, 128), np.float32)
        wc = np.zeros((96, 2, 128), np.float32)
        for hh in range(2):
            oc = slice(hh * 128, (hh + 1) * 128)
            for b3, (dy, dx) in enumerate(X_TAPS_A):
                wx1[32 * b3:32 * b3 + 32, hh, :] = wx_f[oc, :, dy, dx].T
            for b3, (dy, dx) in enumerate(X_TAPS_B):
                wx2[32 * b3:32 * b3 + 32, hh, :] = wx_f[oc, :, dy, dx].T
            for dy in range(3):
                for b3 in range(2):
                    wa[64 * b3:64 * b3 + 64, dy, hh, :] = wh_f[oc, :, dy, b3].T
            wb2[0:64, hh, :] = wh_f[oc, :, 0, 2].T
            wb2[64:128, hh, :] = wh_f[oc, :, 2, 2].T
            wc[0:64, hh, :] = wh_f[oc, :, 1, 2].T
            wc[64:96, hh, :] = wx_f[oc, :, 2, 2].T
        packed_w[parity] = tuple(
            np.ascontiguousarray(a.reshape(a.shape[0], -1).astype(np_dtm))
            for a in (wx1, wx2, wa, wb2, wc))

    for core in range(8):
        b, parity = core // 2, core % 2
        xv = x[:, b]
        if parity == 1:
            xv = xv[:, :, ::-1, :]
        xp = np.zeros((T, C, HP, WP), np.float32)
        xp[:, :, 1:49, 1:65] = xv[:, :, 0:48, :]
        wx1, wx2, wa, wb2, wc = packed_w[parity]
        in_maps.append({
            "xp": np.ascontiguousarray(xp.reshape(T, C, FLAT).astype(np_dtm)),
            "wx1": wx1, "wx2": wx2, "wa": wa, "wb2": wb2, "wc": wc,
            "bias": bias,
        })
    return in_maps


def kernel(x, w_x2h, b_x2h, w_h2h, b_h2h, _trace=False, _tmpdir=None):
    from concourse.bass_utils import run_bass_kernel_spmd

    if "nc" not in _CACHE:
        _CACHE["nc"] = _build_program()
    nc = _CACHE["nc"]

    in_maps = _host_prep(x, w_x2h, b_x2h, w_h2h, b_h2h)
    kw = {}
    if _trace:
        kw = dict(trace=True, tmpdir=_tmpdir)
    res = run_bass_kernel_spmd(nc, in_maps, core_ids=list(range(8)), **kw)

    full = np.zeros((T, B, HID, H, W), np.float32)
    for core in range(8):
        b, parity = core // 2, core % 2
        out = res.results[core]["out"].reshape(T, HID, 32, 66)[:, :, :, 1:65]
        if parity == 0:
            full[:, b, :, 0:32] = out
        else:
            full[:, b, :, 32:64] = out[:, :, ::-1, :]
    if _trace:
        return full, res
    return full
